# revision 1
# baseline (speedup 1.0000x reference)
"""RNN-T joint network kernel for 8 Trainium2 NeuronCores.

Reference computation:
    enc_proj = enc_out @ W_enc.T + b_enc          # [B,T,J]
    dec_proj = pred_out @ W_dec.T + b_dec         # [B,U,J]
    joint    = tanh(enc_proj[:,:,None,:] + dec_proj[:,None,:,:])
    out      = joint @ W_out.T + b_out            # [B,T,U,V]

Shapes (hardcoded): B=4, T=256, U=128, D=512, J=640, V=1024.

Sharding: data-parallel over the B*T = 1024 encoder rows; core k gets
batch b=k//2 and t-range [(k%2)*128, (k%2)*128+128).  Each core computes
its full [128, 128, 1024] output slab.

On-device layout (per core): everything is kept j-major ("transposed")
so that the J=640 contraction of the dominant GEMM lands on the PE
partition axis:
    encP[j, t]  (5 chunks of 128 j)   decP[j, u] (+ combined bias)
    jointT[j, (t,u)] = tanh(decP[j,u] + encP[j,t])   (DVE bcast-add + ACT tanh)
    out[(t,u), v] = jointT.T @ W_outT (+ b_out via DVE on PSUM->SBUF copy)
"""

import os
import numpy as np

B, T, U, D, J, V = 4, 256, 128, 512, 640, 1024
NCORES = 8
TC = (B * T) // NCORES          # 128 t-rows per core
JC = J // 128                   # 5 j-chunks
DC = D // 128                   # 4 d-chunks
G = 8                            # t-rows per lattice group
NG = TC // G                    # 16 groups

# matmul dtype for the dominant GEMM: "bfloat16", "float32", "float32r"
MAIN_DT_NAME = os.environ.get("TRNK_DT", "bfloat16")

_CACHE = {}


def _build_bass():
    import concourse.mybir as mybir
    import concourse.tile as tile
    import concourse.bacc as bacc

    f32 = mybir.dt.float32
    main_dt = getattr(mybir.dt, MAIN_DT_NAME)
    # projections / DVE-visible tiles: bf16 in bf16 mode, else plain f32
    bf16_mode = MAIN_DT_NAME == "bfloat16"
    proj_dt = mybir.dt.bfloat16 if bf16_mode else f32
    lat_dt = proj_dt            # dtype of decP/encP/pre tiles (DVE ops)

    nc = bacc.Bacc("TRN2", debug=False)

    enc_d = nc.dram_tensor("enct", [D, TC], proj_dt, kind="ExternalInput")
    pred_d = nc.dram_tensor("predt", [D, U], proj_dt, kind="ExternalInput")
    wenc_d = nc.dram_tensor("wenct", [D, J], proj_dt, kind="ExternalInput")
    wdec_d = nc.dram_tensor("wdect", [D, J], proj_dt, kind="ExternalInput")
    wout_d = nc.dram_tensor("woutt", [J, V], main_dt, kind="ExternalInput")
    bcomb_d = nc.dram_tensor("bcomb", [128, JC], f32, kind="ExternalInput")
    bout_d = nc.dram_tensor("boutr", [128, V], f32, kind="ExternalInput")
    out_d = nc.dram_tensor("out", [TC, U, V], f32, kind="ExternalOutput")

    enc_ap, pred_ap = enc_d.ap(), pred_d.ap()
    wenc_ap, wdec_ap, wout_ap = wenc_d.ap(), wdec_d.ap(), wout_d.ap()
    out_ap = out_d.ap()

    Tanh = mybir.ActivationFunctionType.Tanh
    Add = mybir.AluOpType.add

    with tile.TileContext(nc) as tc:
        with (
            tc.tile_pool(name="consts", bufs=1) as consts,
            tc.tile_pool(name="proj", bufs=1) as proj,
            tc.tile_pool(name="joint", bufs=2 * JC) as jointp,
            tc.tile_pool(name="osb", bufs=6) as osbp,
            tc.tile_pool(name="psB", bufs=4, space="PSUM") as psB,
        ):
            # ---- load inputs; projection operands first so PE can start ----
            enc_t, pred_t, wenc_t, wdec_t = [], [], [], []
            for dc in range(DC):
                sl = slice(dc * 128, (dc + 1) * 128)
                a = consts.tile([128, TC], proj_dt, tag=f"enc{dc}")
                nc.sync.dma_start(a[:], enc_ap[sl, :])
                enc_t.append(a)
                p = consts.tile([128, U], proj_dt, tag=f"pred{dc}")
                nc.sync.dma_start(p[:], pred_ap[sl, :])
                pred_t.append(p)
                we = consts.tile([128, J], proj_dt, tag=f"wenc{dc}")
                nc.sync.dma_start(we[:], wenc_ap[sl, :])
                wenc_t.append(we)
                wd = consts.tile([128, J], proj_dt, tag=f"wdec{dc}")
                nc.sync.dma_start(wd[:], wdec_ap[sl, :])
                wdec_t.append(wd)

            bcomb_t = consts.tile([128, JC], f32, tag="bcomb")
            nc.sync.dma_start(bcomb_t[:], bcomb_d.ap()[:])
            wout_t = []
            for c in range(JC):
                w = consts.tile([128, V], main_dt, tag=f"wout{c}")
                nc.sync.dma_start(w[:], wout_ap[c * 128:(c + 1) * 128, :])
                wout_t.append(w)
            bout_t = consts.tile([128, V], f32, tag="bout")
            nc.sync.dma_start(bout_t[:], bout_d.ap()[:])

            # ---- projections: encP[c][j, t], decP[c][j, u] (bias folded) ----
            encP, decP = [], []
            for c in range(JC):
                jsl = slice(c * 128, (c + 1) * 128)
                pse = psB.tile([128, TC], f32, tag="ps")
                for dc in range(DC):
                    nc.tensor.matmul(pse[:], wenc_t[dc][:, jsl], enc_t[dc][:],
                                     start=(dc == 0), stop=(dc == DC - 1))
                # encP stays f32: ScalarE's bias operand must be f32.
                # Copies ride DVE so ACT's FIFO is free for early tanh ops.
                e = proj.tile([128, TC], f32, tag=f"encP{c}")
                nc.vector.tensor_copy(e[:], pse[:])
                encP.append(e)

                psd = psB.tile([128, U], f32, tag="ps")
                for dc in range(DC):
                    nc.tensor.matmul(psd[:], wdec_t[dc][:, jsl], pred_t[dc][:],
                                     start=(dc == 0), stop=(dc == DC - 1))
                d = proj.tile([128, U], lat_dt, tag=f"decP{c}")
                nc.vector.tensor_scalar_add(d[:], psd[:], bcomb_t[:, c:c + 1])
                decP.append(d)

            # ---- main loop over t-groups ----
            for g in range(NG):
                # joint[j, (i,u)] = tanh(decP[j,u] + encP[j,t]) — the
                # broadcast-add rides ScalarE's per-partition bias port.
                # Emit t-major so each t's matmuls unlock after JC ACT ops,
                # not after (JC-1)*G+1 of them.
                joint_t = []
                jview = []
                for c in range(JC):
                    jt = jointp.tile([128, G * U], main_dt, tag="joint")
                    joint_t.append(jt)
                    jview.append(jt[:] if main_dt == proj_dt
                                 else jt.bitcast(proj_dt)[:])
                for i in range(G):
                    t = g * G + i
                    for c in range(JC):
                        nc.scalar.activation(
                            jview[c][:, i * U:(i + 1) * U], decP[c][:], Tanh,
                            bias=encP[c][:, t:t + 1])

                for i in range(G):
                    t = g * G + i
                    usl = slice(i * U, (i + 1) * U)
                    osb = osbp.tile([128, V], f32, tag="osb")
                    ps = psB.tile([128, V], f32, tag="ps")
                    for v in range(2):
                        vsl = slice(v * 512, (v + 1) * 512)
                        for c in range(JC):
                            nc.tensor.matmul(ps[:, vsl], joint_t[c][:, usl],
                                             wout_t[c][:, vsl],
                                             start=(c == 0), stop=(c == JC - 1))
                    if g == NG - 1 and i >= G - 2:
                        for v in range(2):
                            vsl = slice(v * 512, (v + 1) * 512)
                            nc.vector.tensor_add(osb[:, vsl], ps[:, vsl],
                                                 bout_t[:, vsl])
                            nc.sync.dma_start(out_ap[t][:, vsl], osb[:, vsl])
                    else:
                        nc.vector.tensor_add(osb[:], ps[:], bout_t[:])
                        nc.sync.dma_start(out_ap[t], osb[:])

    nc.compile()
    return nc


def _host_prep(enc_out, pred_out, W_enc, b_enc, W_dec, b_dec, W_out, b_out):
    import concourse.mybir as mybir
    main_np = np.dtype(mybir.dt.np(getattr(mybir.dt, MAIN_DT_NAME)))
    proj_np = main_np if MAIN_DT_NAME == "bfloat16" else np.dtype(np.float32)

    wencT = np.ascontiguousarray(np.asarray(W_enc, np.float32).T).astype(proj_np)
    wdecT = np.ascontiguousarray(np.asarray(W_dec, np.float32).T).astype(proj_np)
    woutT = np.ascontiguousarray(np.asarray(W_out, np.float32).T).astype(main_np)
    bcomb = np.ascontiguousarray(
        (np.asarray(b_enc, np.float32) + np.asarray(b_dec, np.float32))
        .reshape(JC, 128).T)
    boutr = np.ascontiguousarray(
        np.broadcast_to(np.asarray(b_out, np.float32), (128, V)))

    in_maps = []
    for k in range(NCORES):
        b, th = k // 2, (k % 2) * TC
        encT = np.ascontiguousarray(
            np.asarray(enc_out[b, th:th + TC], np.float32).T).astype(proj_np)
        predT = np.ascontiguousarray(
            np.asarray(pred_out[b], np.float32).T).astype(proj_np)
        in_maps.append({
            "enct": encT, "predt": predT, "wenct": wencT, "wdect": wdecT,
            "woutt": woutT, "bcomb": bcomb, "boutr": boutr,
        })
    return in_maps


def kernel(enc_out, pred_out, W_enc, b_enc, W_dec, b_dec, W_out, b_out):
    from concourse import bass_utils

    if "nc" not in _CACHE:
        _CACHE["nc"] = _build_bass()
    nc = _CACHE["nc"]

    in_maps = _host_prep(enc_out, pred_out, W_enc, b_enc, W_dec, b_dec,
                         W_out, b_out)

    trace = bool(int(os.environ.get("TRNK_PROFILE", "0")))
    res = bass_utils.run_bass_kernel_spmd(
        nc, in_maps, core_ids=list(range(NCORES)), trace=trace)
    kernel.last_exec_ns = res.exec_time_ns

    full = np.empty((B, T, U, V), np.float32)
    for k in range(NCORES):
        b, th = k // 2, (k % 2) * TC
        full[b, th:th + TC] = res.results[k]["out"]
    return full


kernel.last_exec_ns = None

